# revision 1
# baseline (speedup 1.0000x reference)
"""AxialAttention (width=False) with the dominant qkv 1x1-conv matmul executed
data-parallel across 8 TRN2 NeuronCores (fp32r tensor-engine matmuls), and the
remaining attention arithmetic on host.

Sharding: batch N=16 -> 2 images per core. Each core computes
qkv[o, (b,h)] = w_qkv @ x_b for its shard (6.6 GFLOP/core of the 62.7 GFLOP
total; the qkv projection is 84% of all FLOPs in this module).
"""
import sys, os

sys.path.insert(0, "/opt/trn_rl_repo")
_DIR = os.path.dirname(os.path.abspath(__file__))
if _DIR not in sys.path:
    sys.path.insert(0, _DIR)

import numpy as np

IN_PLANES = 512
OUT_PLANES = 512
GROUPS = 8
K = 56
GP = OUT_PLANES // GROUPS
N = 16
EPS = 1e-5
NCORE = 8
P = 128
F = (N // NCORE) * K * K          # per-core (b,h) columns = 6272
O2 = 2 * OUT_PLANES               # 1024

_CACHE = {}


def _split_waits(nc, mybir, limit=1):
    ctr = 0
    for bb in nc.main_func.blocks:
        insts = list(bb.instructions)
        newlist = []
        changed = False
        for ins in insts:
            si = ins.sync_info
            ow = list(si.on_wait) if si is not None and si.on_wait else []
            if len(ow) > limit:
                changed = True
                excess, keep = ow[:-limit], ow[-limit:]
                for i in range(0, len(excess), limit):
                    ctr += 1
                    nop = mybir.InstNoOp(name=f"WSPLIT-{ctr}", ins=[], outs=[])
                    nop.engine = ins.engine
                    nop.sync_info = mybir.SyncInfo(on_wait=list(excess[i:i + limit]),
                                                   on_update=[])
                    nc.register_instruction(nop, overwrite=True)
                    newlist.append(nop)
                ins.sync_info = mybir.SyncInfo(
                    on_wait=list(keep),
                    on_update=list(si.on_update) if si.on_update else [])
            newlist.append(ins)
        if changed:
            bb.instructions = newlist
    return ctr


def _build():
    import concourse.bass as bass
    import concourse.mybir as mybir
    import concourse.tile as tile
    F32 = mybir.dt.float32
    F32R = mybir.dt.float32r
    AF = mybir.ActivationFunctionType

    nc = bass.Bass("TRN2", target_bir_lowering=False, debug=False, num_devices=NCORE)
    X_d = nc.declare_dram_parameter("xin", [IN_PLANES, F], F32, isOutput=False)
    W_d = nc.declare_dram_parameter("wqkv", [IN_PLANES, O2], F32, isOutput=False)
    Y_d = nc.declare_dram_parameter("qkv", [O2, F], F32, isOutput=True)

    NCH = 14
    FCH = F // NCH  # 448

    with tile.TileContext(nc, num_cores=NCORE) as tc:
        with (
            tc.tile_pool(name="const", bufs=1) as const,
            tc.tile_pool(name="xin", bufs=4) as xin,
            tc.tile_pool(name="outp", bufs=4) as outp,
            tc.tile_pool(name="ps", bufs=4, space="PSUM") as ps,
        ):
            w = const.tile([P, 4, O2], F32R)
            wf = xin.tile([P, 4, O2], F32, tag="wld")
            nc.sync.dma_start(wf[:], W_d.ap().rearrange("(ko p) o -> p ko o", p=P))
            nc.vector.tensor_copy(w[:], wf[:])
            for ch in range(NCH):
                xf = xin.tile([P, 4, FCH], F32, tag="xf")
                nc.sync.dma_start(
                    xf[:],
                    X_d.ap().rearrange("(ko p) f -> p ko f", p=P)[:, :, ch * FCH:(ch + 1) * FCH])
                xr = xin.tile([P, 4, FCH], F32R, tag="xr")
                nc.vector.tensor_copy(xr[:], xf[:])
                for m in range(O2 // P):
                    pt = ps.tile([P, FCH], F32, tag="qkvp")
                    for k in range(4):
                        nc.tensor.matmul(pt[:], w[:, k, m * P:(m + 1) * P], xr[:, k],
                                         start=(k == 0), stop=(k == 3))
                    ot = outp.tile([P, FCH], F32, tag="ot")
                    nc.scalar.activation(ot[:], pt[:], AF.Copy)
                    nc.sync.dma_start(
                        Y_d.ap()[m * P:(m + 1) * P, ch * FCH:(ch + 1) * FCH], ot[:])
    _split_waits(nc, mybir, 1)
    return nc


def _get_nc():
    if "nc" not in _CACHE:
        _CACHE["nc"] = _build()
    return _CACHE["nc"]


def _run_device_qkv(x):
    """x: [N, C, K, K] f32 -> qkv [N*K(w), O2, K(h)] f32 via 8-core SPMD."""
    from concourse import bass_utils
    nc = _get_nc()
    npc = N // NCORE
    in_maps = []
    for c in range(NCORE):
        xs = x[c * npc:(c + 1) * npc]                    # [2, C, H, W]
        xt = np.ascontiguousarray(xs.transpose(1, 0, 3, 2).reshape(IN_PLANES, F))
        in_maps.append({"xin": xt, "wqkv": _CACHE["wT"]})
    res = bass_utils.run_bass_kernel_spmd(nc, in_maps, core_ids=list(range(NCORE)))
    _CACHE["last_exec_ns"] = res.exec_time_ns
    out = np.empty((N * K, O2, K), np.float32)
    for c in range(NCORE):
        q = res.results[c]["qkv"]                        # [O2, (b,h)] b=(n_l,w)
        out[c * npc * K:(c + 1) * npc * K] = (
            q.reshape(O2, npc * K, K).transpose(1, 0, 2))
    return out


def kernel(x, w_qkv, relative, g_qkv, b_qkv, g_sim, b_sim, g_out, b_out):
    x = np.asarray(x, np.float32)
    w_qkv = np.asarray(w_qkv, np.float32)
    relative = np.asarray(relative, np.float32)
    g_qkv = np.asarray(g_qkv, np.float32); b_qkv = np.asarray(b_qkv, np.float32)
    g_sim = np.asarray(g_sim, np.float32); b_sim = np.asarray(b_sim, np.float32)
    g_out = np.asarray(g_out, np.float32); b_out = np.asarray(b_out, np.float32)

    _CACHE["wT"] = np.ascontiguousarray(w_qkv.T)         # [C, O2]

    # ---- device: qkv projection (84% of FLOPs), data-parallel over N ----
    qkv = _run_device_qkv(x)                             # [b=N*W, O2, H]

    # ---- host: BN + axial attention (matches reference bit-for-bit in fp32) ----
    b = qkv.shape[0]
    mean = qkv.mean(axis=(0, 2), keepdims=True)
    var = qkv.var(axis=(0, 2), keepdims=True)
    qkvn = (qkv - mean) / np.sqrt(var + EPS) * g_qkv.reshape(1, -1, 1) + b_qkv.reshape(1, -1, 1)
    qkvn = qkvn.reshape(b, GROUPS, 2 * GP, K)
    q = qkvn[:, :, :GP // 2]
    k = qkvn[:, :, GP // 2:GP]
    v = qkvn[:, :, GP:]

    qi = np.arange(K)[None, :]
    ki = np.arange(K)[:, None]
    flat_idx = (ki - qi + K - 1).reshape(-1)
    all_emb = relative[:, flat_idx].reshape(2 * GP, K, K)
    q_emb = all_emb[:GP // 2]
    k_emb = all_emb[GP // 2:GP]
    v_emb = all_emb[GP:]

    qr = np.einsum("bgci,cij->bgij", q, q_emb, optimize=True)
    kr = np.einsum("bgci,cij->bgij", k, k_emb, optimize=True).transpose(0, 1, 3, 2)
    qk = np.einsum("bgci,bgcj->bgij", q, k, optimize=True)
    stacked = np.concatenate([qk, qr, kr], axis=1)
    sm = stacked.mean(axis=(0, 2, 3), keepdims=True)
    sv_ = stacked.var(axis=(0, 2, 3), keepdims=True)
    stacked = (stacked - sm) / np.sqrt(sv_ + EPS) * g_sim.reshape(1, -1, 1, 1) + b_sim.reshape(1, -1, 1, 1)
    sim = stacked.reshape(b, 3, GROUPS, K, K).sum(axis=1)
    sim = sim - sim.max(axis=3, keepdims=True)
    np.exp(sim, out=sim)
    sim /= sim.sum(axis=3, keepdims=True)
    svv = np.einsum("bgij,bgcj->bgci", sim, v, optimize=True)
    sve = np.einsum("bgij,cij->bgci", sim, v_emb, optimize=True)
    out = np.concatenate([svv, sve], axis=-1).reshape(b, 2 * OUT_PLANES, K)
    om = out.mean(axis=(0, 2), keepdims=True)
    ov = out.var(axis=(0, 2), keepdims=True)
    out = (out - om) / np.sqrt(ov + EPS) * g_out.reshape(1, -1, 1) + b_out.reshape(1, -1, 1)
    out = out.reshape(N, K, OUT_PLANES, 2, K).sum(axis=3)
    return np.ascontiguousarray(out.transpose(0, 2, 3, 1)).astype(np.float32)



# revision 2
# speedup vs baseline: 1.1853x; 1.1853x over previous
"""AxialAttention (width=False): the qkv 1x1-conv projection (84% of module
FLOPs) runs on 8 TRN2 NeuronCores; BatchNorms + axial attention run on host
in fp32.

Sharding: data-parallel over batch N=16 -> 2 images per core. Per core the
device computes qkv[1024, 6272] = w[512,1024].T @ x[512,6272] as 448 bf16
matmuls (fp32 PSUM accumulate) streamed at the tensor-engine roofline:

- bf16 x/w upload and bf16 qkv download halve HBM traffic vs fp32 (20.3 MB
  per core) so the kernel is tensor-bound, not DMA-bound.
- Chunk-pairs share one weight load per two matmuls, which lets the PE hide
  LDWEIGHTS behind the matmul stream (189 ns/MM at 2.4 GHz = issue roofline).
- PSUM->SBUF copies alternate between the scalar and vector engines so
  neither becomes the bottleneck; outputs DMA out as [128, 896] tiles.
- Initial x/w DMAs are split per contraction slab and interleaved so the
  matmul stream starts as soon as the first slab lands; warmup matmuls on a
  zeroed tile keep the PE HAM activity window busy during the load phase.
"""
import sys, os

sys.path.insert(0, "/opt/trn_rl_repo")

import numpy as np
import ml_dtypes

BF16 = ml_dtypes.bfloat16
IN_PLANES = 512
OUT_PLANES = 512
GROUPS = 8
K = 56
GP = OUT_PLANES // GROUPS
N = 16
EPS = 1e-5
NCORE = 8
P = 128
F = (N // NCORE) * K * K          # per-core (b,h) columns = 6272
O2 = 2 * OUT_PLANES               # 1024

_CACHE = {}


def _split_waits(nc, mybir, limit=1):
    ctr = 0
    for bb in nc.main_func.blocks:
        insts = list(bb.instructions)
        newlist = []
        changed = False
        for ins in insts:
            si = ins.sync_info
            ow = list(si.on_wait) if si is not None and si.on_wait else []
            if len(ow) > limit:
                changed = True
                excess, keep = ow[:-limit], ow[-limit:]
                for i in range(0, len(excess), limit):
                    ctr += 1
                    nop = mybir.InstNoOp(name=f"WSPLIT-{ctr}", ins=[], outs=[])
                    nop.engine = ins.engine
                    nop.sync_info = mybir.SyncInfo(on_wait=list(excess[i:i + limit]),
                                                   on_update=[])
                    nc.register_instruction(nop, overwrite=True)
                    newlist.append(nop)
                ins.sync_info = mybir.SyncInfo(
                    on_wait=list(keep),
                    on_update=list(si.on_update) if si.on_update else [])
            newlist.append(ins)
        if changed:
            bb.instructions = newlist
    return ctr


def _build():
    import concourse.bass as bass
    import concourse.mybir as mybir
    import concourse.tile as tile
    F32 = mybir.dt.float32
    BF = mybir.dt.bfloat16

    nc = bass.Bass("TRN2", target_bir_lowering=False, debug=False, num_devices=NCORE)
    X_d = nc.declare_dram_parameter("xin", [IN_PLANES, F], BF, isOutput=False)
    W_d = nc.declare_dram_parameter("wqkv", [IN_PLANES, O2], BF, isOutput=False)
    Y_d = nc.declare_dram_parameter("qkv", [O2, F], BF, isOutput=True)

    NPAIR = 7
    FP2 = F // NPAIR   # 896 per chunk-pair
    FCH = FP2 // 2     # 448 per psum tile

    with tile.TileContext(nc, num_cores=NCORE) as tc:
        with (
            tc.tile_pool(name="const", bufs=1) as const,
            tc.tile_pool(name="xin", bufs=3) as xin,
            tc.tile_pool(name="outp", bufs=6) as outp,
            tc.tile_pool(name="ps", bufs=3, space="PSUM") as ps,
        ):
            # Initial loads interleaved per contraction slab (ko) so the k=0
            # matmuls can start once x0[ko=0] + w[ko=0] land.
            w = const.tile([P, 4, O2], BF)
            wr = W_d.ap().rearrange("(ko p) o -> p ko o", p=P)
            xr = X_d.ap().rearrange("(ko p) f -> p ko f", p=P)
            xf0 = xin.tile([P, 4, FP2], BF, tag="xf")
            for k in range(4):
                nc.sync.dma_start(xf0[:, k], xr[:, k, 0:FP2])
                nc.sync.dma_start(w[:, k], wr[:, k])

            # PE warmup during the load phase: matmuls on a zeroed tile keep
            # the HAM activity window busy so the real stream starts warm.
            wu = const.tile([P, P], BF)
            nc.vector.memset(wu[:], 0.0)
            wups = ps.tile([P, P], F32, tag="wu", bufs=1)
            for _ in range(24):
                nc.tensor.matmul(wups[:], wu[:], wu[:], start=True, stop=True)

            for cp in range(NPAIR):
                if cp == 0:
                    xf = xf0
                else:
                    xf = xin.tile([P, 4, FP2], BF, tag="xf")
                    nc.sync.dma_start(xf[:], xr[:, :, cp * FP2:(cp + 1) * FP2])
                for m in range(O2 // P):
                    pa = ps.tile([P, FCH], F32, tag="pa")
                    pb = ps.tile([P, FCH], F32, tag="pb")
                    for k in range(4):
                        lw = w[:, k, m * P:(m + 1) * P]
                        nc.tensor.matmul(pa[:], lw, xf[:, k, :FCH],
                                         start=(k == 0), stop=(k == 3))
                        nc.tensor.matmul(pb[:], lw, xf[:, k, FCH:],
                                         start=(k == 0), stop=(k == 3))
                    ot = outp.tile([P, FP2], BF, tag="ot")
                    if m % 2 == 0:
                        nc.scalar.activation(ot[:, :FCH], pa[:],
                                             mybir.ActivationFunctionType.Copy)
                        nc.scalar.activation(ot[:, FCH:], pb[:],
                                             mybir.ActivationFunctionType.Copy)
                    else:
                        nc.vector.tensor_copy(ot[:, :FCH], pa[:])
                        nc.vector.tensor_copy(ot[:, FCH:], pb[:])
                    nc.sync.dma_start(
                        Y_d.ap()[m * P:(m + 1) * P, cp * FP2:(cp + 1) * FP2], ot[:])
    _split_waits(nc, mybir, 1)
    return nc


def _get_nc():
    if "nc" not in _CACHE:
        _CACHE["nc"] = _build()
    return _CACHE["nc"]


def _make_in_maps(x):
    npc = N // NCORE
    in_maps = []
    for c in range(NCORE):
        xs = x[c * npc:(c + 1) * npc]                    # [2, C, H, W]
        xt = np.ascontiguousarray(
            xs.transpose(1, 0, 3, 2).reshape(IN_PLANES, F)).astype(BF16)
        in_maps.append({"xin": xt, "wqkv": _CACHE["wT"]})
    return in_maps


def _run_device_qkv(x):
    """x: [N, C, K, K] f32 -> qkv [N*K(w), O2, K(h)] f32 via 8-core SPMD."""
    from concourse import bass_utils
    nc = _get_nc()
    npc = N // NCORE
    res = bass_utils.run_bass_kernel_spmd(nc, _make_in_maps(x),
                                          core_ids=list(range(NCORE)))
    _CACHE["last_exec_ns"] = res.exec_time_ns
    out = np.empty((N * K, O2, K), np.float32)
    for c in range(NCORE):
        q = res.results[c]["qkv"].astype(np.float32)     # [O2, (b,h)] b=(n_l,w)
        out[c * npc * K:(c + 1) * npc * K] = (
            q.reshape(O2, npc * K, K).transpose(1, 0, 2))
    return out


def kernel(x, w_qkv, relative, g_qkv, b_qkv, g_sim, b_sim, g_out, b_out):
    x = np.asarray(x, np.float32)
    w_qkv = np.asarray(w_qkv, np.float32)
    relative = np.asarray(relative, np.float32)
    g_qkv = np.asarray(g_qkv, np.float32); b_qkv = np.asarray(b_qkv, np.float32)
    g_sim = np.asarray(g_sim, np.float32); b_sim = np.asarray(b_sim, np.float32)
    g_out = np.asarray(g_out, np.float32); b_out = np.asarray(b_out, np.float32)

    _CACHE["wT"] = np.ascontiguousarray(w_qkv.T).astype(BF16)     # [C, O2]

    # ---- device: qkv projection (84% of FLOPs), data-parallel over N ----
    qkv = _run_device_qkv(x)                             # [b=N*W, O2, H]

    # ---- host: BN + axial attention in fp32 (batched BLAS matmuls) ----
    b = qkv.shape[0]
    bg = b * GROUPS
    mean = qkv.mean(axis=(0, 2), keepdims=True)
    var = qkv.var(axis=(0, 2), keepdims=True)
    qkvn = (qkv - mean) / np.sqrt(var + EPS) * g_qkv.reshape(1, -1, 1) + b_qkv.reshape(1, -1, 1)
    qkvn = qkvn.reshape(b, GROUPS, 2 * GP, K)
    q = qkvn[:, :, :GP // 2]
    k = qkvn[:, :, GP // 2:GP]
    v = qkvn[:, :, GP:]

    qi = np.arange(K)[None, :]
    ki = np.arange(K)[:, None]
    flat_idx = (ki - qi + K - 1).reshape(-1)
    all_emb = relative[:, flat_idx].reshape(2 * GP, K, K)
    q_emb = all_emb[:GP // 2]
    k_emb = all_emb[GP // 2:GP]
    v_emb = all_emb[GP:]

    c2 = GP // 2
    # qr[b,g,i,j] = sum_c q[b,g,c,i] q_emb[c,i,j]  (batched over i)
    qr = np.matmul(q.transpose(3, 0, 1, 2).reshape(K, bg, c2),
                   q_emb.transpose(1, 0, 2))
    qr = qr.reshape(K, b, GROUPS, K).transpose(1, 2, 0, 3)
    # kr[b,g,i,j] = sum_c k[b,g,c,j] k_emb[c,j,i]  (einsum then transpose)
    kr = np.matmul(k.transpose(3, 0, 1, 2).reshape(K, bg, c2),
                   k_emb.transpose(1, 0, 2))
    kr = kr.reshape(K, b, GROUPS, K).transpose(1, 2, 3, 0)
    qk = np.matmul(q.transpose(0, 1, 3, 2), k)
    stacked = np.concatenate([qk, qr, kr], axis=1)
    sm = stacked.mean(axis=(0, 2, 3), keepdims=True)
    sv_ = stacked.var(axis=(0, 2, 3), keepdims=True)
    stacked = (stacked - sm) / np.sqrt(sv_ + EPS) * g_sim.reshape(1, -1, 1, 1) + b_sim.reshape(1, -1, 1, 1)
    sim = stacked.reshape(b, 3, GROUPS, K, K).sum(axis=1)
    sim = sim - sim.max(axis=3, keepdims=True)
    np.exp(sim, out=sim)
    sim /= sim.sum(axis=3, keepdims=True)
    # sv[b,g,c,i] = sum_j sim[b,g,i,j] v[b,g,c,j]
    svv = np.matmul(v, sim.transpose(0, 1, 3, 2))
    # sve[b,g,c,i] = sum_j sim[b,g,i,j] v_emb[c,i,j]  (batched over i)
    sve = np.matmul(sim.transpose(2, 0, 1, 3).reshape(K, bg, K),
                    v_emb.transpose(1, 2, 0))
    sve = sve.reshape(K, b, GROUPS, GP).transpose(1, 2, 3, 0)
    out = np.concatenate([svv, sve], axis=-1).reshape(b, 2 * OUT_PLANES, K)
    om = out.mean(axis=(0, 2), keepdims=True)
    ov = out.var(axis=(0, 2), keepdims=True)
    out = (out - om) / np.sqrt(ov + EPS) * g_out.reshape(1, -1, 1) + b_out.reshape(1, -1, 1)
    out = out.reshape(N, K, OUT_PLANES, 2, K).sum(axis=3)
    return np.ascontiguousarray(out.transpose(0, 2, 3, 1)).astype(np.float32)


# revision 4
# speedup vs baseline: 1.1879x; 1.0023x over previous
"""AxialAttention (width=False): the qkv 1x1-conv projection (84% of module
FLOPs) runs on 8 TRN2 NeuronCores; BatchNorms + axial attention run on host
in fp32.

Sharding: data-parallel over batch N=16 -> 2 images per core. Per core the
device computes qkv[1024, 6272] = w[512,1024].T @ x[512,6272] as 448 bf16
matmuls (fp32 PSUM accumulate) streamed at the tensor-engine roofline:

- bf16 x/w upload and bf16 qkv download halve HBM traffic vs fp32 (20.3 MB
  per core) so the kernel is tensor-bound, not DMA-bound.
- Chunk-pairs share one weight load per two matmuls, which lets the PE hide
  LDWEIGHTS behind the matmul stream (189 ns/MM at 2.4 GHz = issue roofline).
- PSUM->SBUF copies alternate between the scalar and vector engines so
  neither becomes the bottleneck; outputs DMA out as [128, 896] tiles.
- Initial x/w DMAs are split per contraction slab and interleaved so the
  matmul stream starts as soon as the first slab lands; warmup matmuls on a
  zeroed tile keep the PE HAM activity window busy during the load phase.
"""
import sys, os

sys.path.insert(0, "/opt/trn_rl_repo")

import numpy as np
import ml_dtypes

BF16 = ml_dtypes.bfloat16
IN_PLANES = 512
OUT_PLANES = 512
GROUPS = 8
K = 56
GP = OUT_PLANES // GROUPS
N = 16
EPS = 1e-5
NCORE = 8
P = 128
F = (N // NCORE) * K * K          # per-core (b,h) columns = 6272
O2 = 2 * OUT_PLANES               # 1024

_CACHE = {}


def _split_waits(nc, mybir, limit=1):
    ctr = 0
    for bb in nc.main_func.blocks:
        insts = list(bb.instructions)
        newlist = []
        changed = False
        for ins in insts:
            si = ins.sync_info
            ow = list(si.on_wait) if si is not None and si.on_wait else []
            if len(ow) > limit:
                changed = True
                excess, keep = ow[:-limit], ow[-limit:]
                for i in range(0, len(excess), limit):
                    ctr += 1
                    nop = mybir.InstNoOp(name=f"WSPLIT-{ctr}", ins=[], outs=[])
                    nop.engine = ins.engine
                    nop.sync_info = mybir.SyncInfo(on_wait=list(excess[i:i + limit]),
                                                   on_update=[])
                    nc.register_instruction(nop, overwrite=True)
                    newlist.append(nop)
                ins.sync_info = mybir.SyncInfo(
                    on_wait=list(keep),
                    on_update=list(si.on_update) if si.on_update else [])
            newlist.append(ins)
        if changed:
            bb.instructions = newlist
    return ctr


def _build():
    import concourse.bass as bass
    import concourse.mybir as mybir
    import concourse.tile as tile
    F32 = mybir.dt.float32
    BF = mybir.dt.bfloat16

    nc = bass.Bass("TRN2", target_bir_lowering=False, debug=False, num_devices=NCORE)
    X_d = nc.declare_dram_parameter("xin", [IN_PLANES, F], BF, isOutput=False)
    W_d = nc.declare_dram_parameter("wqkv", [IN_PLANES, O2], BF, isOutput=False)
    Y_d = nc.declare_dram_parameter("qkv", [O2, F], BF, isOutput=True)

    NPAIR = 7
    FP2 = F // NPAIR   # 896 per chunk-pair
    FCH = FP2 // 2     # 448 per psum tile

    with tile.TileContext(nc, num_cores=NCORE) as tc:
        with (
            tc.tile_pool(name="const", bufs=1) as const,
            tc.tile_pool(name="xin", bufs=3) as xin,
            tc.tile_pool(name="outp", bufs=12) as outp,
            tc.tile_pool(name="ps", bufs=3, space="PSUM") as ps,
        ):
            # Initial loads interleaved per contraction slab (ko) so the k=0
            # matmuls can start once x0[ko=0] + w[ko=0] land.
            w = const.tile([P, 4, O2], BF)
            wr = W_d.ap().rearrange("(ko p) o -> p ko o", p=P)
            xr = X_d.ap().rearrange("(ko p) f -> p ko f", p=P)
            xf0 = xin.tile([P, 4, FP2], BF, tag="xf")
            for k in range(4):
                nc.sync.dma_start(xf0[:, k], xr[:, k, 0:FP2])
                nc.sync.dma_start(w[:, k], wr[:, k])

            # PE warmup during the load phase: matmuls on a zeroed tile keep
            # the HAM activity window busy so the real stream starts warm.
            # All PSUM goes through one 8-slot tag so pair 0 can hold 8
            # accumulators at once (k-outer ramp below).
            wu = const.tile([P, P], BF)
            nc.vector.memset(wu[:], 0.0)
            wups = ps.tile([P, P], F32, tag="ps8", bufs=8)
            for _ in range(24):
                nc.tensor.matmul(wups[:], wu[:], wu[:], start=True, stop=True)

            # Pair 0 runs k-OUTER in two half-passes: after just x0[k]+w[k]
            # land, all 8 m-groups' k-th matmuls are runnable (16 MMs of work
            # per arrived slab instead of 2), eliminating arrival-pacing
            # stalls during the ramp. Each half-pass holds 8 PSUM banks.
            ots0 = []
            for m in range(O2 // P):
                ots0.append(outp.tile([P, FP2], BF, tag="ot", name=f"ot0_{m}"))
            for half in range(2):
                sl = slice(half * FCH, (half + 1) * FCH)
                pts = [ps.tile([P, FCH], F32, tag="ps8", bufs=8,
                               name=f"p0_{half}_{m}")
                       for m in range(O2 // P)]
                for k in range(4):
                    for m in range(O2 // P):
                        nc.tensor.matmul(pts[m][:], w[:, k, m * P:(m + 1) * P],
                                         xf0[:, k, sl],
                                         start=(k == 0), stop=(k == 3))
                for m in range(O2 // P):
                    if m % 2 == 0:
                        nc.scalar.activation(ots0[m][:, sl], pts[m][:],
                                             mybir.ActivationFunctionType.Copy)
                    else:
                        nc.vector.tensor_copy(ots0[m][:, sl], pts[m][:])
            for m in range(O2 // P):
                nc.sync.dma_start(Y_d.ap()[m * P:(m + 1) * P, 0:FP2], ots0[m][:])

            for cp in range(1, NPAIR):
                xf = xin.tile([P, 4, FP2], BF, tag="xf")
                nc.sync.dma_start(xf[:], xr[:, :, cp * FP2:(cp + 1) * FP2])
                for m in range(O2 // P):
                    pa = ps.tile([P, FCH], F32, tag="ps8", bufs=8)
                    pb = ps.tile([P, FCH], F32, tag="ps8", bufs=8)
                    for k in range(4):
                        lw = w[:, k, m * P:(m + 1) * P]
                        nc.tensor.matmul(pa[:], lw, xf[:, k, :FCH],
                                         start=(k == 0), stop=(k == 3))
                        nc.tensor.matmul(pb[:], lw, xf[:, k, FCH:],
                                         start=(k == 0), stop=(k == 3))
                    ot = outp.tile([P, FP2], BF, tag="ot")
                    if m % 2 == 0:
                        nc.scalar.activation(ot[:, :FCH], pa[:],
                                             mybir.ActivationFunctionType.Copy)
                        nc.scalar.activation(ot[:, FCH:], pb[:],
                                             mybir.ActivationFunctionType.Copy)
                    else:
                        nc.vector.tensor_copy(ot[:, :FCH], pa[:])
                        nc.vector.tensor_copy(ot[:, FCH:], pb[:])
                    nc.sync.dma_start(
                        Y_d.ap()[m * P:(m + 1) * P, cp * FP2:(cp + 1) * FP2], ot[:])
    _split_waits(nc, mybir, 1)
    return nc


def _get_nc():
    if "nc" not in _CACHE:
        _CACHE["nc"] = _build()
    return _CACHE["nc"]


def _make_in_maps(x):
    npc = N // NCORE
    in_maps = []
    for c in range(NCORE):
        xs = x[c * npc:(c + 1) * npc]                    # [2, C, H, W]
        xt = np.ascontiguousarray(
            xs.transpose(1, 0, 3, 2).reshape(IN_PLANES, F)).astype(BF16)
        in_maps.append({"xin": xt, "wqkv": _CACHE["wT"]})
    return in_maps


def _run_device_qkv(x):
    """x: [N, C, K, K] f32 -> qkv [N*K(w), O2, K(h)] f32 via 8-core SPMD."""
    from concourse import bass_utils
    nc = _get_nc()
    npc = N // NCORE
    res = bass_utils.run_bass_kernel_spmd(nc, _make_in_maps(x),
                                          core_ids=list(range(NCORE)))
    _CACHE["last_exec_ns"] = res.exec_time_ns
    out = np.empty((N * K, O2, K), np.float32)
    for c in range(NCORE):
        q = res.results[c]["qkv"].astype(np.float32)     # [O2, (b,h)] b=(n_l,w)
        out[c * npc * K:(c + 1) * npc * K] = (
            q.reshape(O2, npc * K, K).transpose(1, 0, 2))
    return out


def kernel(x, w_qkv, relative, g_qkv, b_qkv, g_sim, b_sim, g_out, b_out):
    x = np.asarray(x, np.float32)
    w_qkv = np.asarray(w_qkv, np.float32)
    relative = np.asarray(relative, np.float32)
    g_qkv = np.asarray(g_qkv, np.float32); b_qkv = np.asarray(b_qkv, np.float32)
    g_sim = np.asarray(g_sim, np.float32); b_sim = np.asarray(b_sim, np.float32)
    g_out = np.asarray(g_out, np.float32); b_out = np.asarray(b_out, np.float32)

    _CACHE["wT"] = np.ascontiguousarray(w_qkv.T).astype(BF16)     # [C, O2]

    # ---- device: qkv projection (84% of FLOPs), data-parallel over N ----
    qkv = _run_device_qkv(x)                             # [b=N*W, O2, H]

    # ---- host: BN + axial attention in fp32 (batched BLAS matmuls) ----
    b = qkv.shape[0]
    bg = b * GROUPS
    mean = qkv.mean(axis=(0, 2), keepdims=True)
    var = qkv.var(axis=(0, 2), keepdims=True)
    qkvn = (qkv - mean) / np.sqrt(var + EPS) * g_qkv.reshape(1, -1, 1) + b_qkv.reshape(1, -1, 1)
    qkvn = qkvn.reshape(b, GROUPS, 2 * GP, K)
    q = qkvn[:, :, :GP // 2]
    k = qkvn[:, :, GP // 2:GP]
    v = qkvn[:, :, GP:]

    qi = np.arange(K)[None, :]
    ki = np.arange(K)[:, None]
    flat_idx = (ki - qi + K - 1).reshape(-1)
    all_emb = relative[:, flat_idx].reshape(2 * GP, K, K)
    q_emb = all_emb[:GP // 2]
    k_emb = all_emb[GP // 2:GP]
    v_emb = all_emb[GP:]

    c2 = GP // 2
    # qr[b,g,i,j] = sum_c q[b,g,c,i] q_emb[c,i,j]  (batched over i)
    qr = np.matmul(q.transpose(3, 0, 1, 2).reshape(K, bg, c2),
                   q_emb.transpose(1, 0, 2))
    qr = qr.reshape(K, b, GROUPS, K).transpose(1, 2, 0, 3)
    # kr[b,g,i,j] = sum_c k[b,g,c,j] k_emb[c,j,i]  (einsum then transpose)
    kr = np.matmul(k.transpose(3, 0, 1, 2).reshape(K, bg, c2),
                   k_emb.transpose(1, 0, 2))
    kr = kr.reshape(K, b, GROUPS, K).transpose(1, 2, 3, 0)
    qk = np.matmul(q.transpose(0, 1, 3, 2), k)
    stacked = np.concatenate([qk, qr, kr], axis=1)
    sm = stacked.mean(axis=(0, 2, 3), keepdims=True)
    sv_ = stacked.var(axis=(0, 2, 3), keepdims=True)
    stacked = (stacked - sm) / np.sqrt(sv_ + EPS) * g_sim.reshape(1, -1, 1, 1) + b_sim.reshape(1, -1, 1, 1)
    sim = stacked.reshape(b, 3, GROUPS, K, K).sum(axis=1)
    sim = sim - sim.max(axis=3, keepdims=True)
    np.exp(sim, out=sim)
    sim /= sim.sum(axis=3, keepdims=True)
    # sv[b,g,c,i] = sum_j sim[b,g,i,j] v[b,g,c,j]
    svv = np.matmul(v, sim.transpose(0, 1, 3, 2))
    # sve[b,g,c,i] = sum_j sim[b,g,i,j] v_emb[c,i,j]  (batched over i)
    sve = np.matmul(sim.transpose(2, 0, 1, 3).reshape(K, bg, K),
                    v_emb.transpose(1, 2, 0))
    sve = sve.reshape(K, b, GROUPS, GP).transpose(1, 2, 3, 0)
    out = np.concatenate([svv, sve], axis=-1).reshape(b, 2 * OUT_PLANES, K)
    om = out.mean(axis=(0, 2), keepdims=True)
    ov = out.var(axis=(0, 2), keepdims=True)
    out = (out - om) / np.sqrt(ov + EPS) * g_out.reshape(1, -1, 1) + b_out.reshape(1, -1, 1)
    out = out.reshape(N, K, OUT_PLANES, 2, K).sum(axis=3)
    return np.ascontiguousarray(out.transpose(0, 2, 3, 1)).astype(np.float32)
